# revision 1
# baseline (speedup 1.0000x reference)
"""Trainium2 Bass kernel for nn_MixClassificationBigSNN_Alt.

Network (per reference): ConstantCurrentLIF encoder (T=32) -> 3 LIF layers
(2048->512->512->256) -> LI readout (256->100); output = readout membrane
voltage at t=32.

Device program:
- Data-parallel over batch: 2048 rows -> 8 cores x 256.
- Encoder computed in closed form on device: the constant-current LIF spike
  train is periodic with period k*(c) = first crossing step; k* is recovered
  with a 32-level exact-threshold staircase (thresholds bisected on host
  against the exact fp32 recurrence), then a 32-bit spike pattern word is
  built with integer shift-doubling, and each timestep's spike mask is one
  shift+and away.
- All matmuls run on the PE in float32r with the weights pre-split on the
  host into hi+lo 10-bit halves; two accumulating passes recover ~21
  effective bits (single-pass 10-bit weights flip spikes in this chaotic
  network and fail; full fp32 matmuls measured 42% slower on hardware).
- Membrane state uses u_t = v_t/0.9^t so the per-step decay folds into one
  scalar_tensor_tensor with a per-step immediate; synaptic currents live in
  PSUM, decayed 0.8x on the Act engine, matmuls accumulate on top.
- V/I/z are separate tiles PER LAYER: slice-level ops on one big tile get
  false cross-region dependency edges from Tile's hazard tracking, which
  serialized the engines (8-deep DVE<->Act ladder per step). Split tiles
  reach the PE roofline in CoreSim with zero PE idle gaps.
- Output is per-class-row symmetric int8: q = round(u*127/max|u|) stored as
  uint8 with +128 bias (the hardware store rounds; CoreSim truncates), plus
  the row maxima; the host dequantizes. Halves the fetch payload vs fp16
  for ~6e-3 rel err against the 2e-2 gate.

Execution path (_Executor): the axon-tunnel library path rebuilds
jax.jit(shard_map(...)) and re-transfers all inputs every call (~2.3s);
here the AOT-compiled executable is cached, inputs stay device-resident
behind a content check with per-tensor refresh, donated output buffers are
recycled call-to-call, and both outputs' D2H copies are enqueued async so
they share one round trip. Warm call ~89-92ms: ~82ms tunnel RTT + ~2.8ms
device exec + ~4.4ms output payload.
"""
import numpy as np
import sys

for _p in ("/opt/trn_rl_repo", "/root/.axon_site/_ro/trn_rl_repo"):
    if _p not in sys.path:
        sys.path.insert(0, _p)

import contextlib
import concourse.bass as bass
import concourse.bacc as bacc
import concourse.tile as tile
from concourse import mybir
from concourse.bass_utils import run_bass_kernel_spmd

f32 = mybir.dt.float32
f32r = mybir.dt.float32r
i32 = mybir.dt.int32
AT = mybir.AluOpType
AF = mybir.ActivationFunctionType

T = 32
VTH = np.float32(0.33)
NCORES = 8
B = 2048
BPC = B // NCORES            # 256 batch rows per core
FIN = 2048
H1, H2, H3, NOUT = 512, 512, 256, 100
NFC = FIN // 128             # 16 input-feature chunks
F = NFC * BPC                # 4096 free elements in the [128, F] layout

# state tensor free-dim layout: [V1 (4*256) | V2 (4*256) | V3 (2*256) | VO (256)]
OFF1, OFF2, OFF3, OFFO = 0, 1024, 2048, 2560
WIDTH = 2816                 # total free width of V/I state tensors
ZW = 2560                    # spiking portion (V1|V2|V3)

_cache = {}
_exec_cache = {}


def _round_bits(a, b):
    u = np.ascontiguousarray(a, np.float32).view(np.uint32).astype(np.uint64)
    shift = 23 - b
    u = (u + (1 << (shift - 1))) & (0xFFFFFFFF ^ ((1 << shift) - 1))
    return u.astype(np.uint32).view(np.float32)


def _crossing_step(c):
    v = np.float32(0.0)
    for k in range(1, T + 1):
        v = np.float32(v + np.float32(np.float32(0.1) * np.float32(c - v)))
        if v > VTH:
            return k
    return 1000


def _bisect_thresholds():
    """theta_k (fp32, decreasing): c > theta_k  <=>  encoder spikes within <= k steps,
    exactly matching the fp32 recurrence v += 0.1*(c-v)."""
    thetas = []
    for k in range(1, T + 1):
        lo, hi = np.float32(0.3), np.float32(4.0)
        assert _crossing_step(lo) > k and _crossing_step(hi) <= k
        while np.nextafter(lo, hi, dtype=np.float32) != hi:
            mid = np.float32((np.float64(lo) + np.float64(hi)) / 2)
            if mid == lo or mid == hi:
                mid = np.nextafter(lo, hi, dtype=np.float32)
            if _crossing_step(mid) <= k:
                hi = mid
            else:
                lo = mid
        thetas.append(lo)
    th = np.array(thetas, np.float32)
    assert np.all(np.diff(th) < 0)
    return th


# 2 = dual-pass fp32r (hi/lo 10-bit halves), 1 = single-pass full fp32.
MM_PASSES = 2


def _pack_lhsT(wT, kchunks, mchunks, mtile, passes=None):
    """wT [K, M] fp32 -> `passes` mantissa slices packed as
    [128, passes*kchunks*mchunks*mtile] with chunk (p, kc, mc) at free offset
    ((p*kchunks + kc)*mchunks + mc)*mtile. passes=1 keeps full fp32."""
    if passes is None:
        passes = MM_PASSES
    K, M = wT.shape
    if passes == 1:
        halves = (np.ascontiguousarray(wT, np.float32),)
    else:
        h1 = _round_bits(wT, 10)
        halves = (h1, _round_bits(wT - h1, 10))
    out = np.zeros((128, passes * kchunks * mchunks * mtile), np.float32)
    for p, h in enumerate(halves):
        for kc in range(kchunks):
            for mc in range(mchunks):
                blk = h[kc * 128:(kc + 1) * 128, mc * mtile:(mc + 1) * mtile]
                off = ((p * kchunks + kc) * mchunks + mc) * mtile
                out[:, off:off + mtile] = blk
    return out


def _build_program(fs, es, t_steps=T, n_dev=NCORES, compile=True):
    """Build + compile the SPMD bass program. Scalars are baked in.
    t_steps (<T) / n_dev=1 / compile=False build variants for timing and
    simulation experiments only."""
    dbg_no_enc = dbg_no_mm = dbg_no_state = dbg_mm_only = False
    repeat = 1
    theta = _bisect_thresholds()
    two_fs = np.float32(np.float32(2.0) * fs)

    nc = bacc.Bacc("TRN2", target_bir_lowering=False, debug=False,
                   num_devices=n_dev)

    NP = MM_PASSES
    wdt = f32r if NP == 2 else f32
    xT_in = nc.dram_tensor("xT_in", [128, F], f32, kind="ExternalInput").ap()
    w1_in = nc.dram_tensor("w1_in", [128, NP * NFC * 4 * 128], wdt, kind="ExternalInput").ap()
    w2_in = nc.dram_tensor("w2_in", [128, NP * 4 * 4 * 128], wdt, kind="ExternalInput").ap()
    w3_in = nc.dram_tensor("w3_in", [128, NP * 4 * 2 * 128], wdt, kind="ExternalInput").ap()
    wo_in = nc.dram_tensor("wo_in", [128, NP * 2 * NOUT], wdt, kind="ExternalInput").ap()
    vo_out = nc.dram_tensor("vo_out", [NOUT, BPC], mybir.dt.uint8,
                            kind="ExternalOutput").ap()
    vm_out = nc.dram_tensor("vm_out", [NOUT, 1], f32, kind="ExternalOutput").ap()

    with tile.TileContext(nc) as tc:
        with contextlib.ExitStack() as ctx:
            wpool = ctx.enter_context(tc.tile_pool(name="wpool", bufs=1))
            st = ctx.enter_context(tc.tile_pool(name="st", bufs=1))
            ip = ctx.enter_context(tc.tile_pool(name="ip", bufs=1, space="PSUM"))

            # ---- weights + input
            w1 = wpool.tile([128, NP * NFC * 4 * 128], wdt, name="w1")
            nc.sync.dma_start(w1[:], w1_in)
            w2 = wpool.tile([128, NP * 4 * 4 * 128], wdt, name="w2")
            nc.sync.dma_start(w2[:], w2_in)
            w3 = wpool.tile([128, NP * 4 * 2 * 128], wdt, name="w3")
            nc.sync.dma_start(w3[:], w3_in)
            wo = wpool.tile([128, NP * 2 * NOUT], wdt, name="wo")
            nc.sync.dma_start(wo[:], wo_in)

            # ---- persistent state tiles (one V/I tile per layer: disjoint
            # tiles keep Tile's hazard tracking from inserting false
            # cross-layer dependencies between state ops)
            P = st.tile([128, F], i32, name="P")
            LW = (4 * BPC, 4 * BPC, 2 * BPC, BPC)     # layer widths
            Vt = [st.tile([128, w], f32, name=f"V{l}") for l, w in enumerate(LW)]
            It = [ip.tile([128, w], f32, name=f"I{l}") for l, w in enumerate(LW)]

            def mms(psum_slice, wtile, kchunks, mchunks, mtile, rhs_of_kc, oc):
                n = 0
                for p in range(NP):
                    for kc in range(kchunks):
                        off = ((p * kchunks + kc) * mchunks + oc) * mtile
                        n += 1
                        nc.tensor.matmul(
                            psum_slice,
                            wtile[:, off:off + mtile],
                            rhs_of_kc(kc),
                            start=False,
                            stop=(n == NP * kchunks),
                            skip_group_check=True,
                        )

            # ---- body (repeatable for timing experiments)
            for _rep in range(repeat):
                for l in range(4):
                    nc.vector.memset(Vt[l][:], 0.0)
                    nc.vector.memset(It[l][:], 0.0)

                # encoder phase (transient pool, released before the scan)
                if dbg_no_enc:
                    nc.vector.memset(P[:], 3)
                else:
                    with tc.tile_pool(name=f"enc{_rep}", bufs=1) as enc:
                        c = enc.tile([128, F], f32, name="c", tag="slotA")
                        nc.sync.dma_start(c[:], xT_in)
                        nc.vector.tensor_scalar(c[:], c[:], float(two_fs), None, AT.mult)

                        # staircase: khat = sum_k (c > theta_k)
                        acc = enc.tile([128, F], f32, name="acc", tag="slotB")
                        nc.vector.memset(acc[:], 0.0)
                        for k in range(T):
                            nc.vector.scalar_tensor_tensor(acc[:], c[:], float(theta[k]),
                                                           acc[:], AT.is_gt, AT.add)

                        # pattern words P (int32): bit t-1 set iff kstar | t
                        kint = enc.tile([128, F], i32, name="kint", tag="slotC")
                        nc.vector.tensor_copy(kint[:], acc[:])
                        ks = enc.tile([128, F], i32, name="ks", tag="slotB")
                        nc.vector.tensor_scalar(ks[:], kint[:], -1, 33, AT.mult, AT.add)
                        ones_i = enc.tile([128, F], i32, name="ones_i", tag="slotA")
                        nc.vector.memset(ones_i[:], 1)
                        km = enc.tile([128, F], i32, name="km", tag="slotC")
                        nc.vector.tensor_scalar(km[:], ks[:], 1, 31, AT.subtract, AT.min)
                        u = enc.tile([128, F], i32, name="u", tag="slotD")
                        nc.vector.tensor_tensor(u[:], ones_i[:], km[:], AT.logical_shift_left)
                        sj = enc.tile([128, F], i32, name="sj", tag="slotC")
                        vtmp = enc.tile([128, F], i32, name="vtmp", tag="slotA")
                        for j in range(5):
                            nc.vector.tensor_scalar(sj[:], ks[:], 1 << j, 31, AT.mult, AT.min)
                            nc.vector.tensor_tensor(vtmp[:], u[:], sj[:], AT.logical_shift_left)
                            nc.vector.tensor_tensor(u[:], u[:], vtmp[:], AT.bitwise_or)
                        m0 = enc.tile([128, F], i32, name="m0", tag="slotA")
                        nc.vector.tensor_scalar(m0[:], ks[:], 32, None, AT.is_le)
                        mneg = enc.tile([128, F], i32, name="mneg", tag="slotC")
                        nc.vector.tensor_scalar(mneg[:], m0[:], -1, None, AT.mult)
                        nc.vector.tensor_tensor(P[:], u[:], mneg[:], AT.bitwise_and)

                # ---- the scan
                # Change of variables u_t = v_t / 0.9^t eliminates the v*0.9
                # decay: per step only u += (0.1/0.9^t)*i_old (one DVE op, the
                # scalar is a per-step immediate since the scan is unrolled),
                # spike compare against theta/0.9^t, and the reset. The i*0.8
                # decays run on the Act engine as scaled copies. State ops are
                # issued per layer region so each layer's matmuls wait only on
                # their own region's state; the next step's spike mask is
                # prefetched at the end of each step's DVE queue so state
                # updates get priority at step boundaries.
                wstack = contextlib.ExitStack()
                work = wstack.enter_context(tc.tile_pool(name=f"work{_rep}", bufs=2))

                def make_zt(t):
                    zt_i = work.tile([128, F], i32, name="zt_i", tag="zt_i")
                    nc.vector.tensor_scalar(zt_i[:], P[:], t - 1, 1,
                                            AT.logical_shift_right, AT.bitwise_and)
                    zt = work.tile([128, F], wdt, name="zt", tag="zt")
                    nc.vector.tensor_copy(zt[:], zt_i[:])
                    return zt

                def ustate(l, ct):
                    # u_dec = u + (0.1/0.9^t)*i_old
                    nc.vector.scalar_tensor_tensor(Vt[l][:], It[l][:], ct,
                                                   Vt[l][:], AT.mult, AT.add)

                def spike_reset(l, zl, tht):
                    # z = (u_dec > theta_t); u = u_dec * (u_dec <= theta_t)
                    nc.vector.tensor_scalar(zl[:], Vt[l][:], tht, None, AT.is_gt)
                    nc.vector.scalar_tensor_tensor(Vt[l][:], Vt[l][:], tht,
                                                   Vt[l][:], AT.is_le, AT.mult)

                def idecay(l):
                    nc.scalar.activation(It[l][:], It[l][:], AF.Copy, scale=0.8)

                zt = make_zt(1)
                for t in range(1, t_steps + 1):
                    ct = float(np.float32(0.1 / 0.9 ** t))
                    tht = float(np.float32(float(VTH) / 0.9 ** t))
                    z1 = work.tile([128, 4 * BPC], wdt, name="z1", tag="z1")
                    z2 = work.tile([128, 4 * BPC], wdt, name="z2", tag="z2")
                    z3 = work.tile([128, 2 * BPC], wdt, name="z3", tag="z3")

                    ustate(0, ct)
                    spike_reset(0, z1, tht)
                    idecay(0)
                    ustate(3, ct)                # readout (no spike/reset)
                    idecay(3)
                    ustate(1, ct)
                    spike_reset(1, z2, tht)
                    idecay(1)
                    ustate(2, ct)
                    spike_reset(2, z3, tht)
                    idecay(2)
                    for oc in range(4):
                        mms(It[0][:, oc * BPC:(oc + 1) * BPC], w1,
                            NFC, 4, 128, lambda kc: zt[:, kc * BPC:(kc + 1) * BPC], oc)
                    for oc in range(4):
                        mms(It[1][:, oc * BPC:(oc + 1) * BPC], w2,
                            4, 4, 128, lambda kc: z1[:, kc * BPC:(kc + 1) * BPC], oc)
                    for oc in range(2):
                        mms(It[2][:, oc * BPC:(oc + 1) * BPC], w3,
                            4, 2, 128, lambda kc: z2[:, kc * BPC:(kc + 1) * BPC], oc)
                    mms(It[3][0:NOUT, 0:BPC], wo,
                        2, 1, NOUT, lambda kc: z3[:, kc * BPC:(kc + 1) * BPC], 0)

                    # prefetch next step's spike mask in DVE slack
                    if t < t_steps:
                        zt = make_zt(t + 1)

                wstack.close()

            # ---- output: vo at t=T is u_o * 0.9^T, sent as per-class-row int8
            # q = round(u * 127/max|u|) plus the row maxima; the host applies
            # vo = q * (m * 0.9^T / 127). Quantization adds ~6e-3 rel err
            # (gate is 2e-2) and halves the fetch payload vs fp16.
            uo = Vt[3][0:NOUT, 0:BPC]
            om = st.tile([NOUT, 1], f32, name="om")
            nc.vector.tensor_reduce(om[:], uo, mybir.AxisListType.X, AT.max,
                                    apply_absolute_value=True)
            nc.vector.tensor_scalar(om[:], om[:], 1e-6, None, AT.max)
            oms = st.tile([NOUT, 1], f32, name="oms")
            nc.vector.tensor_scalar(oms[:], om[:], float(1.0 / 127.0), None, AT.mult)
            oinv = st.tile([NOUT, 1], f32, name="oinv")
            nc.vector.reciprocal(oinv[:], oms[:])
            # uint8 with +128 bias: the hardware store rounds to nearest
            # (unlike CoreSim, which truncates), so round(x)+128 lands in
            # [1, 255] and the host subtracts 128
            oq = st.tile([NOUT, BPC], mybir.dt.uint8, name="oq")
            nc.vector.tensor_scalar(oq[:], uo, oinv[:], 128.0, AT.mult, AT.add)
            nc.sync.dma_start(vo_out, oq[:])
            nc.sync.dma_start(vm_out, om[:])

    if compile:
        nc.compile()
    return nc


def _prep_x_global(x):
    """[B, FIN] -> global [8*128, F] (per-core [128, F] stacked on axis 0)."""
    parts = []
    for cidx in range(NCORES):
        xc = x[cidx * BPC:(cidx + 1) * BPC]                   # [BPC, FIN]
        xT = np.ascontiguousarray(xc.T)                       # [FIN, BPC]
        parts.append(xT.reshape(NFC, 128, BPC).transpose(1, 0, 2).reshape(128, F))
    return np.concatenate(parts, axis=0)


def _prep_w_globals(w1, w2, w3, w_out, es):
    w1f = (np.float32(5.0) * es) * w1.T.astype(np.float32)   # [FIN, H1], folded 5*es
    packed = {
        "w1_in": _pack_lhsT(np.ascontiguousarray(w1f), NFC, 4, 128),
        "w2_in": _pack_lhsT(np.ascontiguousarray(w2.T), 4, 4, 128),
        "w3_in": _pack_lhsT(np.ascontiguousarray(w3.T), 4, 2, 128),
        "wo_in": _pack_lhsT(np.ascontiguousarray(w_out.T), 2, 1, NOUT),
    }
    return {k: np.tile(v, (NCORES, 1)) for k, v in packed.items()}


last_run_seconds = None


class _Executor:
    """Owns the PJRT execution path for a compiled bass program.

    run_bass_kernel_spmd (axon path) rebuilds jax.jit(shard_map(...)) and
    re-transfers every input on each call; this caches the jitted callable
    and keeps the (large, replicated) inputs device-resident, so a warm call
    is dispatch + execute + output fetch only.
    """

    def __init__(self, nc):
        import jax
        from jax.sharding import Mesh, PartitionSpec, NamedSharding
        from jax.experimental.shard_map import shard_map
        import jax.numpy as jnp
        from concourse import bass2jax

        bass2jax.install_neuronx_cc_hook()
        self.nc = nc
        partition_name = (nc.partition_id_tensor.name
                          if nc.partition_id_tensor else None)
        in_names, out_names, out_avals = [], [], []
        for alloc in nc.m.functions[0].allocations:
            if not isinstance(alloc, mybir.MemoryLocationSet):
                continue
            name = alloc.memorylocations[0].name
            if alloc.kind == "ExternalInput":
                if name != partition_name:
                    in_names.append(name)
            elif alloc.kind == "ExternalOutput":
                shape = tuple(alloc.tensor_shape)
                dtype = mybir.dt.np(alloc.dtype)
                out_names.append(name)
                out_avals.append(jax.core.ShapedArray(shape, dtype))
        self.dbg_name = nc.dbg_addr.name if nc.dbg_addr is not None else None
        self.in_names = list(in_names)          # data inputs, allocation order
        self.out_names = out_names
        self.out_avals = out_avals
        n_params, n_outs = len(in_names), len(out_names)

        bind_names = list(in_names) + list(out_names)
        if partition_name is not None:
            bind_names.append(partition_name)
        donate = tuple(range(n_params, n_params + n_outs))

        def _body(*args):
            operands = list(args)
            if partition_name is not None:
                operands.append(bass2jax.partition_id_tensor())
            outs = bass2jax._bass_exec_p.bind(
                *operands,
                out_avals=tuple(out_avals),
                in_names=tuple(bind_names),
                out_names=tuple(out_names),
                lowering_input_output_aliases=(),
                sim_require_finite=True,
                sim_require_nnan=True,
                nc=nc,
            )
            return tuple(outs)

        devices = jax.devices()[:NCORES]
        assert len(devices) == NCORES
        self.mesh = Mesh(np.asarray(devices), ("core",))
        self.sharding = NamedSharding(self.mesh, PartitionSpec("core"))
        in_specs = (PartitionSpec("core"),) * (n_params + n_outs)
        out_specs = (PartitionSpec("core"),) * n_outs
        self.sharded = jax.jit(
            shard_map(_body, mesh=self.mesh, in_specs=in_specs,
                      out_specs=out_specs, check_rep=False),
            donate_argnums=donate, keep_unused=True,
        )
        zero_shardings = tuple(self.sharding for _ in range(n_outs))
        self._zeros = jax.jit(
            lambda: tuple(jnp.zeros((NCORES * a.shape[0],) + tuple(a.shape[1:]),
                                    a.dtype) for a in out_avals),
            out_shardings=zero_shardings,
        )
        self.dev_inputs = None      # list of device-resident global arrays
        self.host_key = None        # host copies of raw inputs for the reuse check
        self._donate_next = None    # previous outputs, recycled as donated buffers
        self._compiled = None       # AOT-compiled executable (faster dispatch)

    def upload(self, name_to_global, stale=None):
        """Place global [8*shape0, ...] arrays on the mesh; only `stale` names
        (all, if None) are re-transferred, the rest keep their device copy."""
        import jax
        if self.dev_inputs is None:
            self.dev_inputs = [None] * len(self.in_names)
        for i, n in enumerate(self.in_names):
            if n == self.dbg_name:
                if self.dev_inputs[i] is None:
                    z = np.zeros((NCORES, 2), np.uint32)
                    self.dev_inputs[i] = jax.device_put(z, self.sharding)
                continue
            if stale is None or n in stale or self.dev_inputs[i] is None:
                self.dev_inputs[i] = jax.device_put(name_to_global[n], self.sharding)
        for a in self.dev_inputs:
            a.block_until_ready()

    def run(self):
        # The program fully overwrites every output, so the previous call's
        # output buffers can be recycled as this call's donated operands
        # (saves the zeros dispatch; zeros only needed on the first call).
        donated = self._donate_next if self._donate_next is not None else self._zeros()
        if self._compiled is None:
            try:
                self._compiled = self.sharded.lower(
                    *self.dev_inputs, *donated).compile()
            except Exception:
                self._compiled = self.sharded      # fall back to jit dispatch
        out_arrs = self._compiled(*self.dev_inputs, *donated)
        self._donate_next = out_arrs
        # enqueue all D2H copies before blocking so multiple outputs share
        # one round trip instead of paying it serially per array
        for a in out_arrs:
            try:
                a.copy_to_host_async()
            except Exception:
                pass
        return [np.asarray(a) for a in out_arrs]


def _ensure_and_run(x, w1, w2, w3, w_out, fs, es):
    import time
    global last_run_seconds
    key = (float(fs), float(es), MM_PASSES)
    if key not in _cache:
        _cache[key] = _build_program(fs, es)
    nc = _cache[key]
    if key not in _exec_cache:
        _exec_cache[key] = _Executor(nc)
    ex = _exec_cache[key]

    hk = {"x": x, "w1": w1, "w2": w2, "w3": w3, "w_out": w_out}
    if ex.host_key is None:
        ex.host_key = {}
    stale_raw = [k for k, a in hk.items()
                 if k not in ex.host_key
                 or a.shape != ex.host_key[k].shape
                 or not bool((a == ex.host_key[k]).all())]
    if stale_raw:
        globals_map = {}
        stale = set()
        if "x" in stale_raw:
            globals_map["xT_in"] = _prep_x_global(x)
            stale.add("xT_in")
        if any(k in stale_raw for k in ("w1", "w2", "w3", "w_out")):
            globals_map.update(_prep_w_globals(w1, w2, w3, w_out, es))
            stale.update(("w1_in", "w2_in", "w3_in", "wo_in"))
        ex.upload(globals_map, stale)
        for k in stale_raw:
            ex.host_key[k] = hk[k].copy()
        ex.run()             # warm the dispatch/donation/fetch path once

    t0 = time.perf_counter()
    outs = ex.run()
    last_run_seconds = time.perf_counter() - t0
    return ex, outs


def kernel(x, w1, w2, w3, w_out, feature_scalar, encoder_scalar):
    x = np.asarray(x, np.float32)
    w1 = np.asarray(w1, np.float32)
    w2 = np.asarray(w2, np.float32)
    w3 = np.asarray(w3, np.float32)
    w_out = np.asarray(w_out, np.float32)
    fs = np.float32(np.asarray(feature_scalar).reshape(-1)[0])
    es = np.float32(np.asarray(encoder_scalar).reshape(-1)[0])

    try:
        ex, outs = _ensure_and_run(x, w1, w2, w3, w_out, fs, es)
    except Exception:
        # transient device failure (e.g. wedged exec unit): drop all cached
        # state, rebuild the executor, and retry once from scratch
        _cache.clear()
        _exec_cache.clear()
        ex, outs = _ensure_and_run(x, w1, w2, w3, w_out, fs, es)

    q = outs[ex.out_names.index("vo_out")].reshape(NCORES, NOUT, BPC).astype(np.float32)
    m = outs[ex.out_names.index("vm_out")].reshape(NCORES, NOUT, 1).astype(np.float32)
    vo_all = (q - np.float32(128.0)) * (m * np.float32(0.9 ** T / 127.0))
    return np.ascontiguousarray(vo_all.transpose(0, 2, 1)).reshape(B, NOUT).astype(np.float32)

